# revision 1
# baseline (speedup 1.0000x reference)
"""AttentionRNN Trainium2 kernel — 8-core data-parallel SPMD.

Strategy
--------
Batch (2048) is sharded 8 ways (256 rows/core).  The heavy sequential
BiLSTM + attention math is prepared so the device does the final
output-projection GEMM stage per core, with the state recurrences
evaluated host-side in fp32 numpy (bit-matched structure to the
reference).  A key algebraic simplification is used: the attention
scores are `a_proj + (h @ w_s)` where the h-term is constant across the
sequence axis, so softmax is invariant to it — alpha and the context
vector are therefore *independent of the decoder state* and computed
once instead of per decoder step.

Device stage (per core): ys.T = W_aug.T @ h2_aug.T  — a [33,128]^T x
[33,2560] GEMM on TensorE (bias folded in as the 33rd row), tiled to
five N=512 PSUM banks, DVE copy-out, DMA to DRAM.
"""

import numpy as np
from contextlib import ExitStack

EMB = 128
H = 32
B = 2048
S = 256
NOUT = 10
NCORES = 8
BL = B // NCORES  # 256 rows per core
NCOLS = NOUT * BL  # 2560 output columns per core
NT = 512  # matmul free-dim tile
LAST_EXEC_NS = 0


def _sigmoid(x):
    # fp32, numerically-stable
    out = np.empty_like(x)
    pos = x >= 0
    out[pos] = 1.0 / (1.0 + np.exp(-x[pos], dtype=np.float32))
    e = np.exp(x[~pos], dtype=np.float32)
    out[~pos] = e / (1.0 + e)
    return out


def _run_lstm(zin, W_hh):
    # zin: [S, B, 4H] precomputed input projections (+bias); returns hs [S, B, H]
    n = zin.shape[1]
    h = np.zeros((n, H), np.float32)
    c = np.zeros((n, H), np.float32)
    hs = np.empty((zin.shape[0], n, H), np.float32)
    WhhT = np.ascontiguousarray(W_hh.T)
    for t in range(zin.shape[0]):
        z = zin[t] + h @ WhhT
        i = _sigmoid(z[:, :H])
        f = _sigmoid(z[:, H : 2 * H])
        g = np.tanh(z[:, 2 * H : 3 * H])
        o = _sigmoid(z[:, 3 * H :])
        c = f * c + i * g
        h = o * np.tanh(c)
        hs[t] = h
    return hs


def _build_nc():
    import concourse.bass as bass
    import concourse.mybir as mybir

    nc = bass.Bass()
    wh = nc.declare_dram_parameter("wh", [H + 1, EMB + NCOLS], mybir.dt.float32, isOutput=False)
    out = nc.declare_dram_parameter("out", [EMB, NCOLS], mybir.dt.float32, isOutput=True)
    NJ = NCOLS // NT

    with ExitStack() as ctx:
        wht = ctx.enter_context(nc.sbuf_tensor("wht", [H + 1, EMB + NCOLS], mybir.dt.float32))
        ot = ctx.enter_context(nc.sbuf_tensor("ot", [EMB, NCOLS], mybir.dt.float32))
        pss = [ctx.enter_context(nc.psum_tensor(f"ps{j}", [EMB, NT], mybir.dt.float32))
               for j in range(NJ)]
        dsem = ctx.enter_context(nc.semaphore("dsem"))
        msem = ctx.enter_context(nc.semaphore("msem"))
        csem = ctx.enter_context(nc.semaphore("csem"))
        block = ctx.enter_context(nc.Block())

        @block.gpsimd
        def _(g):
            g.dma_start(wht[:, :], wh[:, :]).then_inc(dsem, 16)
            g.wait_ge(csem, NJ)
            g.dma_start(out[:, :], ot[:, :]).then_inc(dsem, 16)
            g.wait_ge(dsem, 32)

        @block.tensor
        def _(t):
            t.wait_ge(dsem, 16)
            for j in range(NJ):
                t.matmul(pss[j][:, :], wht[:, :EMB],
                         wht[:, EMB + j * NT : EMB + (j + 1) * NT],
                         start=True, stop=True).then_inc(msem, 1)

        @block.vector
        def _(v):
            for j in range(NJ):
                v.wait_ge(msem, j + 1)
                v.tensor_copy(ot[:, j * NT : (j + 1) * NT], pss[j][:, :]).then_inc(csem, 1)

    return nc


def kernel(x, n_output, emb, Wf_ih, Wf_hh, bf_ih, bf_hh, Wb_ih, Wb_hh, bb_ih, bb_hh,
           Wd_ih, Wd_hh, bd_ih, bd_hh, w_att, b_att, W_out, b_out):
    import os, time
    os.environ["BASS_NEVER_TRACE"] = "1"  # NTFF hook unavailable in this env
    from concourse.bass_utils import run_bass_kernel_spmd

    x = np.asarray(x)
    n_output = int(n_output)
    f32 = lambda a: np.asarray(a, dtype=np.float32)
    emb, Wf_ih, Wf_hh, Wb_ih, Wb_hh, Wd_ih, Wd_hh, W_out = map(
        f32, (emb, Wf_ih, Wf_hh, Wb_ih, Wb_hh, Wd_ih, Wd_hh, W_out))
    bf = f32(bf_ih) + f32(bf_hh)
    bb = f32(bb_ih) + f32(bb_hh)
    bd = f32(bd_ih) + f32(bd_hh)
    w_att, b_att, b_out = f32(w_att), f32(b_att), f32(b_out)

    # ---- host: embedding + input projections (parallel GEMMs) ----
    xe = emb[x]  # [B, S, H]
    xs = np.swapaxes(xe, 0, 1)  # [S, B, H]
    flat = xs.reshape(-1, H)
    zin_f = (flat @ Wf_ih.T + bf).reshape(S, B, 4 * H)
    zin_b = (np.ascontiguousarray(xs[::-1]).reshape(-1, H) @ Wb_ih.T + bb).reshape(S, B, 4 * H)

    # ---- host: the two sequential LSTM scans ----
    hf = _run_lstm(zin_f, Wf_hh)             # [S, B, H]
    hb = _run_lstm(zin_b, Wb_hh)[::-1]       # [S, B, H]

    # ---- attention: alpha is independent of decoder state (softmax shift
    # invariance over the h @ w_s term), so ctx is computed once ----
    w_a = w_att[H:]
    a_proj = (np.einsum('sbe,e->bs', hf, w_a[:H], dtype=np.float32, casting='same_kind')
              + np.einsum('sbe,e->bs', hb, w_a[H:], dtype=np.float32, casting='same_kind')
              + b_att[0])                     # [B, S]
    m = a_proj.max(axis=1, keepdims=True)
    e = np.exp(a_proj - m, dtype=np.float32)
    alpha = e / e.sum(axis=1, keepdims=True)  # [B, S]
    ctx_f = np.einsum('bs,sbe->be', alpha, hf)
    ctx_b = np.einsum('bs,sbe->be', alpha, hb)
    ctx_v = np.concatenate([ctx_f, ctx_b], axis=1).astype(np.float32)  # [B, 2H]

    # ---- decoder: 10-step recurrence; collect h2_t, defer the output
    # projection (py = h2 @ W_out.T + b_out) to the device GEMM ----
    Wd_py = Wd_ih[:, :EMB]      # [4H, 128]
    Wd_cx = Wd_ih[:, EMB:]      # [4H, 2H]
    zc = ctx_v @ Wd_cx.T + bd   # constant across steps  [B, 4H]
    h = np.zeros((B, H), np.float32)
    c = np.zeros((B, H), np.float32)
    py = np.zeros((B, EMB), np.float32)
    h2s = np.empty((n_output, B, H), np.float32)
    for t in range(n_output):
        z = zc + py @ Wd_py.T + h @ Wd_hh.T
        i = _sigmoid(z[:, :H])
        f = _sigmoid(z[:, H : 2 * H])
        g = np.tanh(z[:, 2 * H : 3 * H])
        o = _sigmoid(z[:, 3 * H :])
        c = f * c + i * g
        h = o * np.tanh(c)
        h2s[t] = h
        py = h @ W_out.T + b_out
    # ---- device: ys = h2_aug @ [W_out.T; b_out] per core ----
    nc = _build_nc()
    w_aug = np.concatenate([W_out.T, b_out[None, :]], axis=0)  # [33, 128]
    in_maps = []
    for k in range(NCORES):
        blk = h2s[:, k * BL : (k + 1) * BL, :]          # [10, BL, 32]
        h2t = blk.reshape(n_output * BL, H).T           # [32, 2560]
        h2t = np.concatenate([h2t, np.ones((1, n_output * BL), np.float32)], axis=0)
        in_maps.append({"wh": np.ascontiguousarray(np.concatenate([w_aug, h2t], axis=1))})
    _t0 = time.time()
    res = run_bass_kernel_spmd(nc, in_maps, list(range(NCORES)))
    outs = res.results
    global LAST_EXEC_NS
    LAST_EXEC_NS = getattr(res, "exec_time_ns", None) or int((time.time() - _t0) * 1e9)
    ys = np.empty((B, n_output, EMB), np.float32)
    for k in range(NCORES):
        o = outs[k]["out"]                               # [128, 2560]
        ys[k * BL : (k + 1) * BL] = o.reshape(EMB, n_output, BL).transpose(2, 1, 0)
    return ys



# revision 3
# speedup vs baseline: 1.6196x; 1.6196x over previous
"""AttentionRNN Trainium2 kernel -- 8-core data-parallel, full on-device model.

Batch (2048) is sharded 8 ways (256 rows/core).  Each core runs the ENTIRE
model on device via one Bass/Tile program:

  embedding lookup   : one-hot trick -- x broadcast (K=1 ones matmul) ->
                       iota compare (DVE) -> table matmul (K=128) against a
                       host-precomputed (emb @ W_ih.T + b) table
  BiLSTM             : 256 fwd + 256 bwd steps packed into shared [*, 512]
                       ops (fwd cols 0:256, bwd 256:512); gate order
                       permuted to i,f,o,g so sigmoid/tanh slices are
                       contiguous; h stored (bf16) 4-steps-per-partition-
                       block: h[s,b,k] at [32*(s%4)+k, (s//4)*256+b]
  attention          : scores via per-partition weight multiply + [128->4]
                       ones matmul; softmax normalization deferred (exp /
                       colsum-Z applied after the context reduction --
                       softmax is shift-invariant wrt the decoder-state
                       term so alpha is decoder-independent and computed
                       once); alpha replicated to the h layout with a fixed
                       [4,128] selector matmul; context = multiply +
                       grouped free reduce + [128->64] combiner matmul
  decoder            : 10 steps; z = Wd_cx@ctx (+bias folded via ones row)
                       + Wd_py@py + Wd_hh@h accumulated in PSUM; output
                       projection W_out/b_out folded the same way; py
                       written straight into the output tile

Two environment workarounds baked in:
  * this walrus build accepts a single sync-wait per instruction, so a BIR
    post-pass splits multi-wait instructions into single-wait NoOps + op
    (installed by monkeypatching bass2jax._decompress_ant_bir);
  * matmul operands/outputs at base partition 32 crash the runtime, so all
    matmuls use base-0 operands (x rows streamed as separate tensors, the
    context partition-combine done as one accumulation group with a
    [128,128] block selector).

kernel() does one untimed warm-up call (compile; NEFF is disk-cached) and
reports LAST_EXEC_NS as the wall time of the subsequent steady-state call.
"""

import numpy as np
from contextlib import ExitStack

EMB = 128
H = 32
VOC = 128
BL = 256
NCORES = 8
B = 2048
S = 256
PERM = np.r_[0:64, 96:128, 64:96]  # gate order i,f,o,g (from i,f,g,o)
LAST_EXEC_NS = 0


def _mk_woff():
    shapes = [("tabf", 128, 128), ("tabb", 128, 128), ("whhf", 32, 128),
              ("whhb", 32, 128), ("wdpy", 128, 128), ("wdcx", 65, 128),
              ("wdhh", 32, 128), ("wout", 33, 128), ("w4", 128, 2),
              ("iotaf", 128, 1), ("cmb2", 128, 128), ("onesg", 128, 4),
              ("sel4b", 4, 128)]
    off, table = 0, {}
    for name, r, c in shapes:
        table[name] = (r, c, off)
        off += c
    return table, off


WOFF, WCOLS = _mk_woff()


def _prep_weights(emb, Wf_ih, Wf_hh, bf, Wb_ih, Wb_hh, bb,
                  Wd_ih, Wd_hh, bd, w_att, W_out, b_out):
    f = lambda a: np.ascontiguousarray(a, dtype=np.float32)
    parts = {}
    parts["tabf"] = f((emb @ Wf_ih.T + bf)[:, PERM])
    parts["tabb"] = f((emb @ Wb_ih.T + bb)[:, PERM])
    parts["whhf"] = f(Wf_hh.T[:, PERM])
    parts["whhb"] = f(Wb_hh.T[:, PERM])
    parts["w4"] = f(np.stack([np.tile(w_att[H:2 * H], 4),
                              np.tile(w_att[2 * H:3 * H], 4)], axis=1))
    parts["wdpy"] = f(Wd_ih[PERM, :EMB].T)
    parts["wdcx"] = f(np.concatenate([Wd_ih[PERM, EMB:].T,
                                      bd[PERM][None, :]], axis=0))
    parts["wdhh"] = f(Wd_hh[PERM].T)
    parts["wout"] = f(np.concatenate([W_out.T, b_out[None, :]], axis=0))
    parts["iotaf"] = np.arange(128, dtype=np.float32)[:, None]
    ones4 = (np.arange(128)[:, None] % 32
             == np.arange(32)[None, :]).astype(np.float32)
    z32 = np.zeros((128, 32), np.float32)
    parts["cmb2"] = np.concatenate([ones4, z32, z32, ones4], axis=1)
    parts["onesg"] = (np.arange(128)[:, None] // 32
                      == np.arange(4)[None, :]).astype(np.float32)
    parts["sel4b"] = (np.arange(4)[:, None]
                      == np.arange(128)[None, :] // 32).astype(np.float32)
    wpack = np.zeros((128, WCOLS), np.float32)
    for name, (rows, cols, off) in WOFF.items():
        wpack[:rows, off:off + cols] = parts[name]
    return wpack


def _prep_xs(x_core):
    import ml_dtypes
    xs2 = np.empty((2, S * BL), np.float32)
    xs2[0] = x_core.T.reshape(-1)
    xs2[1] = x_core[:, ::-1].T.reshape(-1)
    return xs2.astype(ml_dtypes.bfloat16)


def _install_birpatch():
    """Split multi-wait instructions: this walrus accepts one sync-wait per
    instruction, so hoist extras onto single-wait NoOps inserted before it
    on the same engine queue (sequencers execute in order -- equivalent)."""
    import orjson
    from concourse import bass2jax
    if getattr(bass2jax._decompress_ant_bir, "_waitsplit", False):
        return
    orig = bass2jax._decompress_ant_bir
    counter = [0]

    def _split_block(bb):
        out = []
        for ins in bb.get("instructions", []):
            si = ins.get("sync_info") or {}
            waits = si.get("on_wait") or []
            if len(waits) > 1:
                for wx in waits[:-1]:
                    counter[0] += 1
                    out.append({"name": f"I-WSPL{counter[0]}",
                                "opcode": "NoOp",
                                "engine": ins.get("engine"),
                                "ins": [], "outs": [],
                                "debug": ins.get("debug", 0),
                                "sync_info": {"on_wait": [wx],
                                              "on_update": []}})
                si["on_wait"] = [waits[-1]]
            out.append(ins)
        bb["instructions"] = out
        for sub in bb.get("blocks", []) or []:
            _split_block(sub)

    def patched(ant_bir_value):
        raw = orig(ant_bir_value)
        try:
            bir = orjson.loads(raw)
            for fn in bir.get("functions", []):
                for bb in fn.get("blocks", []) or []:
                    _split_block(bb)
            return orjson.dumps(bir)
        except Exception:
            return raw

    patched._waitsplit = True
    bass2jax._decompress_ant_bir = patched


def _build_nc(NOUT=10):
    import concourse.bass as bass
    import concourse.tile as tile
    from concourse import mybir

    f32 = mybir.dt.float32
    bf16 = mybir.dt.bfloat16
    NB = S // 4
    NTHI = NB
    CHUNK = 8
    AF = mybir.ActivationFunctionType

    nc = bass.Bass()
    d_xs = nc.declare_dram_parameter("xs", [2, S * BL], bf16, isOutput=False)
    d_wp = nc.declare_dram_parameter("wpack", [128, WCOLS], f32,
                                     isOutput=False)
    d_out = nc.declare_dram_parameter("out", [EMB, NOUT * BL], f32,
                                      isOutput=True)

    with ExitStack() as ctx:
        tc = ctx.enter_context(tile.TileContext(nc))
        const = ctx.enter_context(tc.tile_pool(name="const", bufs=1))
        state = ctx.enter_context(tc.tile_pool(name="state", bufs=1))
        work = ctx.enter_context(tc.tile_pool(name="work", bufs=2))
        xpool = ctx.enter_context(tc.tile_pool(name="xpool", bufs=2))

        wpack = const.tile([128, WCOLS], f32)
        nc.gpsimd.dma_start(wpack[:, :], d_wp[:, :])
        w = {name: wpack[0:r, off:off + c]
             for name, (r, c, off) in WOFF.items()}
        iota_f = w["iotaf"]
        ones_r = const.tile([1, 128], f32)
        nc.vector.memset(ones_r, 1.0)
        ones_b1 = const.tile([1, 128], bf16)
        nc.vector.memset(ones_b1, 1.0)
        ones_cf = const.tile([128, 1], f32)
        nc.vector.memset(ones_cf, 1.0)
        onesg_b = const.tile([128, 4], bf16)
        nc.vector.tensor_copy(onesg_b, w["onesg"])
        sel4b_b = const.tile([4, 128], bf16)
        nc.vector.tensor_copy(sel4b_b, w["sel4b"])

        # PE pre-touch of wpack: keeps later matmuls at one wait each
        # (LDWEIGHTS carries a single wait slot).  Pool stays open so the
        # PSUM bank is never reused (reuse would add a bank-WAW wait).
        ps_warm = ctx.enter_context(
            tc.tile_pool(name="ps_warm", bufs=1, space="PSUM"))
        warm = ps_warm.tile([1, 1], f32)
        nc.tensor.matmul(warm, wpack[0:1, 0:1], wpack[0:1, 0:1],
                         start=True, stop=True)

        # ---- scan state ----
        hT2 = state.tile([32, 2 * BL], f32)
        nc.vector.memset(hT2, 0.0)
        c2 = state.tile([32, 2 * BL], f32)
        nc.vector.memset(c2, 0.0)
        hf4 = state.tile([128, NB * BL], bf16)
        hb4 = state.tile([128, NB * BL], bf16)

        AL = mybir.AluOpType

        with tc.tile_pool(name="ps_scan", bufs=2, space="PSUM") as ps_scan, \
             tc.tile_pool(name="ps_scan2", bufs=2, space="PSUM") as ps_scan2:
            CHX = 16
            xchf = xchb = None
            for t in range(S):
                sb_ = S - 1 - t
                if t % CHX == 0:
                    xchf = xpool.tile([1, CHX * BL], bf16, tag="xchf")
                    nc.gpsimd.dma_start(xchf[0:1, :],
                                        d_xs[0:1, t * BL:(t + CHX) * BL])
                    xchb = xpool.tile([1, CHX * BL], bf16, tag="xchb")
                    nc.gpsimd.dma_start(xchb[0:1, :],
                                        d_xs[1:2, t * BL:(t + CHX) * BL])
                lt = t % CHX
                psx = ps_scan.tile([128, 2 * BL], f32, tag="psx")
                nc.tensor.matmul(psx[:, 0:BL], ones_b1,
                                 xchf[0:1, lt * BL:(lt + 1) * BL],
                                 start=True, stop=True)
                nc.tensor.matmul(psx[:, BL:2 * BL], ones_b1,
                                 xchb[0:1, lt * BL:(lt + 1) * BL],
                                 start=True, stop=True)
                oh = work.tile([128, 2 * BL], f32, tag="oh")
                nc.vector.tensor_scalar(oh, psx, iota_f, None,
                                        op0=AL.is_equal)

                psz = ps_scan2.tile([128, 2 * BL], f32, tag="psz")
                nc.tensor.matmul(psz[:, 0:BL], w["tabf"], oh[:, 0:BL],
                                 start=True, stop=False)
                nc.tensor.matmul(psz[:, 0:BL], w["whhf"], hT2[:, 0:BL],
                                 start=False, stop=True)
                nc.tensor.matmul(psz[:, BL:2 * BL], w["tabb"],
                                 oh[:, BL:2 * BL], start=True, stop=False)
                nc.tensor.matmul(psz[:, BL:2 * BL], w["whhb"],
                                 hT2[:, BL:2 * BL], start=False, stop=True)

                sgi = work.tile([32, 2 * BL], f32, tag="sgi")
                nc.scalar.activation(sgi, psz[0:32, :], AF.Sigmoid)
                sgf = work.tile([32, 2 * BL], f32, tag="sgf")
                nc.scalar.activation(sgf, psz[32:64, :], AF.Sigmoid)
                sgo = work.tile([32, 2 * BL], f32, tag="sgo")
                nc.scalar.activation(sgo, psz[64:96, :], AF.Sigmoid)
                tg = work.tile([32, 2 * BL], f32, tag="tg")
                nc.scalar.activation(tg, psz[96:128, :], AF.Tanh)
                t1 = work.tile([32, 2 * BL], f32, tag="t1")
                nc.vector.tensor_mul(t1, sgi, tg)
                nc.vector.tensor_mul(c2, sgf, c2)
                nc.vector.tensor_add(c2, c2, t1)
                tnc = work.tile([32, 2 * BL], f32, tag="tnc")
                nc.scalar.activation(tnc, c2, AF.Tanh)
                nc.vector.tensor_mul(hT2, sgo, tnc)

                nc.gpsimd.tensor_copy(
                    hf4[32 * (t % 4):32 * (t % 4) + 32,
                        (t // 4) * BL:(t // 4) * BL + BL], hT2[:, 0:BL])
                nc.gpsimd.tensor_copy(
                    hb4[32 * (sb_ % 4):32 * (sb_ % 4) + 32,
                        (sb_ // 4) * BL:(sb_ // 4) * BL + BL],
                    hT2[:, BL:2 * BL])

        # ---- attention ----
        exp4 = state.tile([4, NB * BL], bf16)
        ctxT = state.tile([65, BL], f32)
        nc.vector.memset(ctxT[64:65, :], 1.0)

        NCH = (NB * BL) // 512
        with tc.tile_pool(name="ps_att", bufs=2, space="PSUM") as ps_att, \
             tc.tile_pool(name="ps_att1", bufs=1, space="PSUM") as ps_att1, \
             tc.tile_pool(name="ps_att2", bufs=2, space="PSUM") as ps_att2, \
             tc.tile_pool(name="att_sb", bufs=2) as att_sb, \
             tc.tile_pool(name="att_acc", bufs=1) as att_acc:
            for ch in range(NCH):
                cs = ch * 512
                whf = att_sb.tile([128, 512], bf16, tag="whf")
                nc.vector.tensor_scalar(whf, hf4[:, cs:cs + 512],
                                        w["w4"][:, 0:1], None, op0=AL.mult)
                whb = att_sb.tile([128, 512], bf16, tag="whb")
                nc.vector.tensor_scalar(whb, hb4[:, cs:cs + 512],
                                        w["w4"][:, 1:2], None, op0=AL.mult)
                s4p = ps_att2.tile([4, 512], f32, tag="s4p")
                nc.tensor.matmul(s4p, onesg_b, whf, start=True, stop=False)
                nc.tensor.matmul(s4p, onesg_b, whb, start=False, stop=True)
                nc.scalar.activation(exp4[:, cs:cs + 512], s4p, AF.Exp)

            zpart = att_acc.tile([4, BL], f32)
            nc.vector.tensor_reduce(
                zpart, exp4.rearrange("p (l b) -> p b l", l=NB),
                axis=mybir.AxisListType.X, op=AL.add)
            zps = ps_att1.tile([1, BL], f32)
            nc.tensor.matmul(zps, ones_cf[0:4, :], zpart,
                             start=True, stop=True)
            zrec = att_acc.tile([1, BL], f32)
            nc.vector.reciprocal(zrec, zps)

            acc_f = att_acc.tile([128, BL], f32)
            acc_b = att_acc.tile([128, BL], f32)
            for ci in range(NTHI // CHUNK):
                tmpf = att_sb.tile([128, CHUNK * BL], bf16, tag="tmpf")
                tmpb = att_sb.tile([128, CHUNK * BL], bf16, tag="tmpb")
                for li in range(CHUNK):
                    thi = ci * CHUNK + li
                    a4 = ps_att.tile([128, BL], f32, tag="a4")
                    nc.tensor.matmul(a4, sel4b_b,
                                     exp4[:, thi * BL:(thi + 1) * BL],
                                     start=True, stop=True)
                    a4s = att_sb.tile([128, BL], bf16, tag="a4s")
                    nc.scalar.activation(a4s, a4, AF.Copy)
                    nc.vector.tensor_mul(tmpf[:, li * BL:(li + 1) * BL],
                                         hf4[:, thi * BL:(thi + 1) * BL],
                                         a4s)
                    nc.vector.tensor_mul(tmpb[:, li * BL:(li + 1) * BL],
                                         hb4[:, thi * BL:(thi + 1) * BL],
                                         a4s)
                for acc, tmp in ((acc_f, tmpf), (acc_b, tmpb)):
                    red = att_sb.tile([128, BL], f32, tag="red")
                    nc.vector.tensor_reduce(
                        red, tmp.rearrange("p (l b) -> p b l", l=CHUNK),
                        axis=mybir.AxisListType.X, op=AL.add)
                    if ci == 0:
                        nc.vector.tensor_copy(acc, red)
                    else:
                        nc.vector.tensor_add(acc, acc, red)

            ctx_ps = ps_att1.tile([64, BL], f32)
            nc.tensor.matmul(ctx_ps, w["cmb2"][:, 0:64], acc_f,
                             start=True, stop=False)
            nc.tensor.matmul(ctx_ps, w["cmb2"][:, 64:128], acc_b,
                             start=False, stop=True)
            zbc = ps_att1.tile([64, BL], f32)
            nc.tensor.matmul(zbc, ones_r[:, 0:64], zrec,
                             start=True, stop=True)
            zbs = att_acc.tile([64, BL], f32)
            nc.vector.tensor_copy(zbs, zbc)
            nc.vector.tensor_mul(ctxT[0:64, :], zbs, ctx_ps)

        # ---- decoder ----
        out_sb = state.tile([EMB, NOUT * BL], f32)
        hTd = state.tile([33, BL], f32)
        nc.vector.memset(hTd, 0.0)
        nc.vector.memset(hTd[32:33, :], 1.0)
        cd = state.tile([32, BL], f32)
        nc.vector.memset(cd, 0.0)

        with tc.tile_pool(name="ps_dec", bufs=2, space="PSUM") as ps_dec, \
             tc.tile_pool(name="dec_sb", bufs=2) as dec_sb:
            for t in range(NOUT):
                zd = ps_dec.tile([128, BL], f32, tag="zd")
                nc.tensor.matmul(zd, w["wdcx"], ctxT,
                                 start=True, stop=(t == 0))
                if t > 0:
                    nc.tensor.matmul(zd, w["wdpy"],
                                     out_sb[:, (t - 1) * BL:t * BL],
                                     start=False, stop=False)
                    nc.tensor.matmul(zd, w["wdhh"], hTd[0:32, :],
                                     start=False, stop=True)
                sdi = dec_sb.tile([32, BL], f32, tag="sdi")
                nc.scalar.activation(sdi, zd[0:32, :], AF.Sigmoid)
                sdf = dec_sb.tile([32, BL], f32, tag="sdf")
                nc.scalar.activation(sdf, zd[32:64, :], AF.Sigmoid)
                sdo = dec_sb.tile([32, BL], f32, tag="sdo")
                nc.scalar.activation(sdo, zd[64:96, :], AF.Sigmoid)
                tgd = dec_sb.tile([32, BL], f32, tag="tgd")
                nc.scalar.activation(tgd, zd[96:128, :], AF.Tanh)
                t1d = dec_sb.tile([32, BL], f32, tag="t1d")
                nc.vector.tensor_mul(t1d, sdi, tgd)
                if t > 0:
                    nc.vector.tensor_mul(cd, sdf, cd)
                    nc.vector.tensor_add(cd, cd, t1d)
                else:
                    nc.vector.tensor_copy(cd, t1d)
                tncd = dec_sb.tile([32, BL], f32, tag="tncd")
                nc.scalar.activation(tncd, cd, AF.Tanh)
                nc.vector.tensor_mul(hTd[0:32, :], sdo, tncd)
                pyp = ps_dec.tile([128, BL], f32, tag="pyp")
                nc.tensor.matmul(pyp, w["wout"], hTd, start=True, stop=True)
                nc.vector.tensor_copy(out_sb[:, t * BL:(t + 1) * BL], pyp)

        nc.gpsimd.dma_start(d_out[:, :], out_sb[:, :])

    return nc


def kernel(x, n_output, emb, Wf_ih, Wf_hh, bf_ih, bf_hh, Wb_ih, Wb_hh,
           bb_ih, bb_hh, Wd_ih, Wd_hh, bd_ih, bd_hh, w_att, b_att,
           W_out, b_out):
    import os, time
    os.environ["BASS_NEVER_TRACE"] = "1"  # no NTFF hook in this env
    _install_birpatch()
    from concourse.bass_utils import run_bass_kernel_spmd

    x = np.asarray(x)
    n_output = int(n_output)
    f32 = lambda a: np.asarray(a, dtype=np.float32)
    wpack = _prep_weights(
        f32(emb), f32(Wf_ih), f32(Wf_hh), f32(bf_ih) + f32(bf_hh),
        f32(Wb_ih), f32(Wb_hh), f32(bb_ih) + f32(bb_hh),
        f32(Wd_ih), f32(Wd_hh), f32(bd_ih) + f32(bd_hh),
        f32(w_att), f32(W_out), f32(b_out))
    nc = _build_nc(NOUT=n_output)

    in_maps = []
    for k in range(NCORES):
        in_maps.append({"wpack": wpack,
                        "xs": _prep_xs(x[k * BL:(k + 1) * BL])})
    cores = list(range(NCORES))

    # warm-up: compiles (NEFF is disk-cached across processes) and primes
    # the transfer path; not part of the reported execution time
    res = None
    _tw0 = time.time()
    for attempt in range(3):
        try:
            res = run_bass_kernel_spmd(nc, in_maps, cores)
            break
        except Exception:
            if attempt == 2:
                raise
            time.sleep(2.0)
    warm_ns = int((time.time() - _tw0) * 1e9)

    # timed steady-state execution
    global LAST_EXEC_NS
    try:
        _t0 = time.time()
        res2 = run_bass_kernel_spmd(nc, in_maps, cores)
        LAST_EXEC_NS = int((time.time() - _t0) * 1e9)
        res = res2
    except Exception:
        LAST_EXEC_NS = warm_ns

    ys = np.empty((B, n_output, EMB), np.float32)
    for k in range(NCORES):
        o = np.asarray(res.results[k]["out"], dtype=np.float32)
        ys[k * BL:(k + 1) * BL] = o.reshape(
            EMB, n_output, BL).transpose(2, 1, 0)
    return ys


# revision 4
# speedup vs baseline: 1.8552x; 1.1455x over previous
"""AttentionRNN Trainium2 kernel -- 8-core data-parallel, full on-device model.

Batch (2048) is sharded 8 ways (256 rows/core).  Each core runs the ENTIRE
model on device via one Bass/Tile program:

  embedding lookup   : one-hot trick -- x broadcast (K=1 ones matmul) ->
                       iota compare (DVE) -> table matmul (K=128) against a
                       host-precomputed (emb @ W_ih.T + b) table
  BiLSTM             : 256 fwd + 256 bwd steps packed into shared [*, 512]
                       ops (fwd cols 0:256, bwd 256:512); gate order
                       permuted to i,f,o,g so sigmoid/tanh slices are
                       contiguous; h stored (bf16) 4-steps-per-partition-
                       block: h[s,b,k] at [32*(s%4)+k, (s//4)*256+b]
  attention          : scores via per-partition weight multiply + [128->4]
                       ones matmul; softmax normalization deferred (exp /
                       colsum-Z applied after the context reduction --
                       softmax is shift-invariant wrt the decoder-state
                       term so alpha is decoder-independent and computed
                       once); alpha replicated to the h layout with a fixed
                       [4,128] selector matmul; context = multiply +
                       grouped free reduce + [128->64] combiner matmul
  decoder            : 10 steps; z = Wd_cx@ctx (+bias folded via ones row)
                       + Wd_py@py + Wd_hh@h accumulated in PSUM; output
                       projection W_out/b_out folded the same way; py
                       written straight into the output tile

Two environment workarounds baked in:
  * this walrus build accepts a single sync-wait per instruction, so a BIR
    post-pass splits multi-wait instructions into single-wait NoOps + op
    (installed by monkeypatching bass2jax._decompress_ant_bir);
  * matmul operands/outputs at base partition 32 crash the runtime, so all
    matmuls use base-0 operands (x rows streamed as separate tensors, the
    context partition-combine done as one accumulation group with a
    [128,128] block selector).

kernel() does one untimed warm-up call (compile; NEFF is disk-cached) and
reports LAST_EXEC_NS as the wall time of the subsequent steady-state call.
"""

import numpy as np
from contextlib import ExitStack

EMB = 128
H = 32
VOC = 128
BL = 256
NCORES = 8
B = 2048
S = 256
PERM = np.r_[0:64, 96:128, 64:96]  # gate order i,f,o,g (from i,f,g,o)
LAST_EXEC_NS = 0


def _mk_woff():
    shapes = [("tabf", 128, 128), ("tabb", 128, 128), ("whhf", 32, 128),
              ("whhb", 32, 128), ("wdpy", 128, 128), ("wdcx", 65, 128),
              ("wdhh", 32, 128), ("wout", 33, 128), ("w4", 128, 2),
              ("iotaf", 128, 1), ("cmb2", 128, 128), ("onesg", 128, 4),
              ("sel4b", 4, 128)]
    off, table = 0, {}
    for name, r, c in shapes:
        table[name] = (r, c, off)
        off += c
    return table, off


WOFF, WCOLS = _mk_woff()


def _prep_weights(emb, Wf_ih, Wf_hh, bf, Wb_ih, Wb_hh, bb,
                  Wd_ih, Wd_hh, bd, w_att, W_out, b_out):
    f = lambda a: np.ascontiguousarray(a, dtype=np.float32)
    parts = {}
    parts["tabf"] = f((emb @ Wf_ih.T + bf)[:, PERM])
    parts["tabb"] = f((emb @ Wb_ih.T + bb)[:, PERM])
    parts["whhf"] = f(Wf_hh.T[:, PERM])
    parts["whhb"] = f(Wb_hh.T[:, PERM])
    parts["w4"] = f(np.stack([np.tile(w_att[H:2 * H], 4),
                              np.tile(w_att[2 * H:3 * H], 4)], axis=1))
    parts["wdpy"] = f(Wd_ih[PERM, :EMB].T)
    parts["wdcx"] = f(np.concatenate([Wd_ih[PERM, EMB:].T,
                                      bd[PERM][None, :]], axis=0))
    parts["wdhh"] = f(Wd_hh[PERM].T)
    parts["wout"] = f(np.concatenate([W_out.T, b_out[None, :]], axis=0))
    parts["iotaf"] = np.arange(128, dtype=np.float32)[:, None]
    ones4 = (np.arange(128)[:, None] % 32
             == np.arange(32)[None, :]).astype(np.float32)
    z32 = np.zeros((128, 32), np.float32)
    parts["cmb2"] = np.concatenate([ones4, z32, z32, ones4], axis=1)
    parts["onesg"] = (np.arange(128)[:, None] // 32
                      == np.arange(4)[None, :]).astype(np.float32)
    parts["sel4b"] = (np.arange(4)[:, None]
                      == np.arange(128)[None, :] // 32).astype(np.float32)
    wpack = np.zeros((128, WCOLS), np.float32)
    for name, (rows, cols, off) in WOFF.items():
        wpack[:rows, off:off + cols] = parts[name]
    return wpack


def _prep_xs(x_core):
    import ml_dtypes
    xs2 = np.empty((2, S * BL), np.float32)
    xs2[0] = x_core.T.reshape(-1)
    xs2[1] = x_core[:, ::-1].T.reshape(-1)
    return xs2.astype(ml_dtypes.bfloat16)


def _install_birpatch():
    """Split multi-wait instructions: this walrus accepts one sync-wait per
    instruction, so hoist extras onto single-wait NoOps inserted before it
    on the same engine queue (sequencers execute in order -- equivalent)."""
    import orjson
    from concourse import bass2jax
    if getattr(bass2jax._decompress_ant_bir, "_waitsplit", False):
        return
    orig = bass2jax._decompress_ant_bir
    counter = [0]

    def _split_block(bb):
        out = []
        for ins in bb.get("instructions", []):
            si = ins.get("sync_info") or {}
            waits = si.get("on_wait") or []
            if len(waits) > 1:
                for wx in waits[:-1]:
                    counter[0] += 1
                    out.append({"name": f"I-WSPL{counter[0]}",
                                "opcode": "NoOp",
                                "engine": ins.get("engine"),
                                "ins": [], "outs": [],
                                "debug": ins.get("debug", 0),
                                "sync_info": {"on_wait": [wx],
                                              "on_update": []}})
                si["on_wait"] = [waits[-1]]
            out.append(ins)
        bb["instructions"] = out
        for sub in bb.get("blocks", []) or []:
            _split_block(sub)

    def patched(ant_bir_value):
        raw = orig(ant_bir_value)
        try:
            bir = orjson.loads(raw)
            for fn in bir.get("functions", []):
                for bb in fn.get("blocks", []) or []:
                    _split_block(bb)
            return orjson.dumps(bir)
        except Exception:
            return raw

    patched._waitsplit = True
    bass2jax._decompress_ant_bir = patched


def _build_nc(NOUT=10):
    import concourse.bass as bass
    import concourse.tile as tile
    from concourse import mybir

    f32 = mybir.dt.float32
    bf16 = mybir.dt.bfloat16
    NB = S // 4
    NTHI = NB
    CHUNK = 8
    AF = mybir.ActivationFunctionType

    nc = bass.Bass()
    d_xs = nc.declare_dram_parameter("xs", [2, S * BL], bf16, isOutput=False)
    d_wp = nc.declare_dram_parameter("wpack", [128, WCOLS], f32,
                                     isOutput=False)
    d_out = nc.declare_dram_parameter("out", [EMB, NOUT * BL], bf16,
                                      isOutput=True)

    with ExitStack() as ctx:
        tc = ctx.enter_context(tile.TileContext(nc))
        const = ctx.enter_context(tc.tile_pool(name="const", bufs=1))
        state = ctx.enter_context(tc.tile_pool(name="state", bufs=1))
        work = ctx.enter_context(tc.tile_pool(name="work", bufs=2))
        xpool = ctx.enter_context(tc.tile_pool(name="xpool", bufs=2))

        wpack = const.tile([128, WCOLS], f32)
        nc.gpsimd.dma_start(wpack[:, :], d_wp[:, :])
        w = {name: wpack[0:r, off:off + c]
             for name, (r, c, off) in WOFF.items()}
        iota_f = w["iotaf"]
        ones_r = const.tile([1, 128], f32)
        nc.vector.memset(ones_r, 1.0)
        ones_b1 = const.tile([1, 128], bf16)
        nc.vector.memset(ones_b1, 1.0)
        ones_cf = const.tile([128, 1], f32)
        nc.vector.memset(ones_cf, 1.0)
        onesg_b = const.tile([128, 4], bf16)
        nc.vector.tensor_copy(onesg_b, w["onesg"])
        sel4b_b = const.tile([4, 128], bf16)
        nc.vector.tensor_copy(sel4b_b, w["sel4b"])
        wdpy_b = const.tile([128, 128], bf16)
        nc.vector.tensor_copy(wdpy_b, w["wdpy"])

        # PE pre-touch of wpack: keeps later matmuls at one wait each
        # (LDWEIGHTS carries a single wait slot).  Pool stays open so the
        # PSUM bank is never reused (reuse would add a bank-WAW wait).
        ps_warm = ctx.enter_context(
            tc.tile_pool(name="ps_warm", bufs=1, space="PSUM"))
        warm = ps_warm.tile([1, 1], f32)
        nc.tensor.matmul(warm, wpack[0:1, 0:1], wpack[0:1, 0:1],
                         start=True, stop=True)

        # ---- scan state ----
        hT2 = state.tile([32, 2 * BL], f32)
        nc.vector.memset(hT2, 0.0)
        c2 = state.tile([32, 2 * BL], f32)
        nc.vector.memset(c2, 0.0)
        hf4 = state.tile([128, NB * BL], bf16)
        hb4 = state.tile([128, NB * BL], bf16)

        AL = mybir.AluOpType

        with tc.tile_pool(name="ps_scan", bufs=2, space="PSUM") as ps_scan, \
             tc.tile_pool(name="ps_scan2", bufs=2, space="PSUM") as ps_scan2:
            CHX = 16
            xchf = xchb = None
            for t in range(S):
                sb_ = S - 1 - t
                if t % CHX == 0:
                    xchf = xpool.tile([1, CHX * BL], bf16, tag="xchf")
                    nc.gpsimd.dma_start(xchf[0:1, :],
                                        d_xs[0:1, t * BL:(t + CHX) * BL])
                    xchb = xpool.tile([1, CHX * BL], bf16, tag="xchb")
                    nc.gpsimd.dma_start(xchb[0:1, :],
                                        d_xs[1:2, t * BL:(t + CHX) * BL])
                lt = t % CHX
                psx = ps_scan.tile([128, 2 * BL], f32, tag="psx")
                nc.tensor.matmul(psx[:, 0:BL], ones_b1,
                                 xchf[0:1, lt * BL:(lt + 1) * BL],
                                 start=True, stop=True)
                nc.tensor.matmul(psx[:, BL:2 * BL], ones_b1,
                                 xchb[0:1, lt * BL:(lt + 1) * BL],
                                 start=True, stop=True)
                oh = work.tile([128, 2 * BL], f32, tag="oh")
                nc.vector.tensor_scalar(oh, psx, iota_f, None,
                                        op0=AL.is_equal)

                psz = ps_scan2.tile([128, 2 * BL], f32, tag="psz")
                nc.tensor.matmul(psz[:, 0:BL], w["tabf"], oh[:, 0:BL],
                                 start=True, stop=False)
                nc.tensor.matmul(psz[:, 0:BL], w["whhf"], hT2[:, 0:BL],
                                 start=False, stop=True)
                nc.tensor.matmul(psz[:, BL:2 * BL], w["tabb"],
                                 oh[:, BL:2 * BL], start=True, stop=False)
                nc.tensor.matmul(psz[:, BL:2 * BL], w["whhb"],
                                 hT2[:, BL:2 * BL], start=False, stop=True)

                sgi = work.tile([32, 2 * BL], f32, tag="sgi")
                nc.scalar.activation(sgi, psz[0:32, :], AF.Sigmoid)
                sgf = work.tile([32, 2 * BL], f32, tag="sgf")
                nc.scalar.activation(sgf, psz[32:64, :], AF.Sigmoid)
                sgo = work.tile([32, 2 * BL], f32, tag="sgo")
                nc.scalar.activation(sgo, psz[64:96, :], AF.Sigmoid)
                tg = work.tile([32, 2 * BL], f32, tag="tg")
                nc.scalar.activation(tg, psz[96:128, :], AF.Tanh)
                t1 = work.tile([32, 2 * BL], f32, tag="t1")
                nc.vector.tensor_mul(t1, sgi, tg)
                nc.vector.tensor_mul(c2, sgf, c2)
                nc.vector.tensor_add(c2, c2, t1)
                tnc = work.tile([32, 2 * BL], f32, tag="tnc")
                nc.scalar.activation(tnc, c2, AF.Tanh)
                nc.vector.tensor_mul(hT2, sgo, tnc)

                nc.gpsimd.tensor_copy(
                    hf4[32 * (t % 4):32 * (t % 4) + 32,
                        (t // 4) * BL:(t // 4) * BL + BL], hT2[:, 0:BL])
                nc.gpsimd.tensor_copy(
                    hb4[32 * (sb_ % 4):32 * (sb_ % 4) + 32,
                        (sb_ // 4) * BL:(sb_ // 4) * BL + BL],
                    hT2[:, BL:2 * BL])

        # ---- attention ----
        exp4 = state.tile([4, NB * BL], bf16)
        ctxT = state.tile([65, BL], f32)
        nc.vector.memset(ctxT[64:65, :], 1.0)

        NCH = (NB * BL) // 512
        with tc.tile_pool(name="ps_att", bufs=2, space="PSUM") as ps_att, \
             tc.tile_pool(name="ps_att1", bufs=1, space="PSUM") as ps_att1, \
             tc.tile_pool(name="ps_att2", bufs=2, space="PSUM") as ps_att2, \
             tc.tile_pool(name="att_sb", bufs=2) as att_sb, \
             tc.tile_pool(name="att_acc", bufs=1) as att_acc:
            for ch in range(NCH):
                cs = ch * 512
                whf = att_sb.tile([128, 512], bf16, tag="whf")
                nc.vector.tensor_scalar(whf, hf4[:, cs:cs + 512],
                                        w["w4"][:, 0:1], None, op0=AL.mult)
                whb = att_sb.tile([128, 512], bf16, tag="whb")
                nc.vector.tensor_scalar(whb, hb4[:, cs:cs + 512],
                                        w["w4"][:, 1:2], None, op0=AL.mult)
                s4p = ps_att2.tile([4, 512], f32, tag="s4p")
                nc.tensor.matmul(s4p, onesg_b, whf, start=True, stop=False)
                nc.tensor.matmul(s4p, onesg_b, whb, start=False, stop=True)
                nc.scalar.activation(exp4[:, cs:cs + 512], s4p, AF.Exp)

            zpart = att_acc.tile([4, BL], f32)
            nc.vector.tensor_reduce(
                zpart, exp4.rearrange("p (l b) -> p b l", l=NB),
                axis=mybir.AxisListType.X, op=AL.add)
            zps = ps_att1.tile([1, BL], f32)
            nc.tensor.matmul(zps, ones_cf[0:4, :], zpart,
                             start=True, stop=True)
            zrec = att_acc.tile([1, BL], f32)
            nc.vector.reciprocal(zrec, zps)

            acc_f = att_acc.tile([128, BL], f32)
            acc_b = att_acc.tile([128, BL], f32)
            for ci in range(NTHI // CHUNK):
                tmpf = att_sb.tile([128, CHUNK * BL], bf16, tag="tmpf")
                tmpb = att_sb.tile([128, CHUNK * BL], bf16, tag="tmpb")
                for li in range(CHUNK):
                    thi = ci * CHUNK + li
                    a4 = ps_att.tile([128, BL], f32, tag="a4")
                    nc.tensor.matmul(a4, sel4b_b,
                                     exp4[:, thi * BL:(thi + 1) * BL],
                                     start=True, stop=True)
                    a4s = att_sb.tile([128, BL], bf16, tag="a4s")
                    nc.scalar.activation(a4s, a4, AF.Copy)
                    nc.vector.tensor_mul(tmpf[:, li * BL:(li + 1) * BL],
                                         hf4[:, thi * BL:(thi + 1) * BL],
                                         a4s)
                    nc.vector.tensor_mul(tmpb[:, li * BL:(li + 1) * BL],
                                         hb4[:, thi * BL:(thi + 1) * BL],
                                         a4s)
                for acc, tmp in ((acc_f, tmpf), (acc_b, tmpb)):
                    red = att_sb.tile([128, BL], f32, tag="red")
                    nc.vector.tensor_reduce(
                        red, tmp.rearrange("p (l b) -> p b l", l=CHUNK),
                        axis=mybir.AxisListType.X, op=AL.add)
                    if ci == 0:
                        nc.vector.tensor_copy(acc, red)
                    else:
                        nc.vector.tensor_add(acc, acc, red)

            ctx_ps = ps_att1.tile([64, BL], f32)
            nc.tensor.matmul(ctx_ps, w["cmb2"][:, 0:64], acc_f,
                             start=True, stop=False)
            nc.tensor.matmul(ctx_ps, w["cmb2"][:, 64:128], acc_b,
                             start=False, stop=True)
            zbc = ps_att1.tile([64, BL], f32)
            nc.tensor.matmul(zbc, ones_r[:, 0:64], zrec,
                             start=True, stop=True)
            zbs = att_acc.tile([64, BL], f32)
            nc.vector.tensor_copy(zbs, zbc)
            nc.vector.tensor_mul(ctxT[0:64, :], zbs, ctx_ps)

        # ---- decoder ----
        out_sb = state.tile([EMB, NOUT * BL], bf16)
        hTd = state.tile([33, BL], f32)
        nc.vector.memset(hTd, 0.0)
        nc.vector.memset(hTd[32:33, :], 1.0)
        cd = state.tile([32, BL], f32)
        nc.vector.memset(cd, 0.0)

        with tc.tile_pool(name="ps_dec", bufs=2, space="PSUM") as ps_dec, \
             tc.tile_pool(name="dec_sb", bufs=2) as dec_sb:
            for t in range(NOUT):
                zd = ps_dec.tile([128, BL], f32, tag="zd")
                nc.tensor.matmul(zd, w["wdcx"], ctxT,
                                 start=True, stop=(t == 0))
                if t > 0:
                    nc.tensor.matmul(zd, wdpy_b,
                                     out_sb[:, (t - 1) * BL:t * BL],
                                     start=False, stop=False)
                    nc.tensor.matmul(zd, w["wdhh"], hTd[0:32, :],
                                     start=False, stop=True)
                sdi = dec_sb.tile([32, BL], f32, tag="sdi")
                nc.scalar.activation(sdi, zd[0:32, :], AF.Sigmoid)
                sdf = dec_sb.tile([32, BL], f32, tag="sdf")
                nc.scalar.activation(sdf, zd[32:64, :], AF.Sigmoid)
                sdo = dec_sb.tile([32, BL], f32, tag="sdo")
                nc.scalar.activation(sdo, zd[64:96, :], AF.Sigmoid)
                tgd = dec_sb.tile([32, BL], f32, tag="tgd")
                nc.scalar.activation(tgd, zd[96:128, :], AF.Tanh)
                t1d = dec_sb.tile([32, BL], f32, tag="t1d")
                nc.vector.tensor_mul(t1d, sdi, tgd)
                if t > 0:
                    nc.vector.tensor_mul(cd, sdf, cd)
                    nc.vector.tensor_add(cd, cd, t1d)
                else:
                    nc.vector.tensor_copy(cd, t1d)
                tncd = dec_sb.tile([32, BL], f32, tag="tncd")
                nc.scalar.activation(tncd, cd, AF.Tanh)
                nc.vector.tensor_mul(hTd[0:32, :], sdo, tncd)
                pyp = ps_dec.tile([128, BL], f32, tag="pyp")
                nc.tensor.matmul(pyp, w["wout"], hTd, start=True, stop=True)
                nc.vector.tensor_copy(out_sb[:, t * BL:(t + 1) * BL], pyp)

        nc.gpsimd.dma_start(d_out[:, :], out_sb[:, :])

    return nc


def kernel(x, n_output, emb, Wf_ih, Wf_hh, bf_ih, bf_hh, Wb_ih, Wb_hh,
           bb_ih, bb_hh, Wd_ih, Wd_hh, bd_ih, bd_hh, w_att, b_att,
           W_out, b_out):
    import os, time
    os.environ["BASS_NEVER_TRACE"] = "1"  # no NTFF hook in this env
    _install_birpatch()
    from concourse.bass_utils import run_bass_kernel_spmd

    x = np.asarray(x)
    n_output = int(n_output)
    f32 = lambda a: np.asarray(a, dtype=np.float32)
    wpack = _prep_weights(
        f32(emb), f32(Wf_ih), f32(Wf_hh), f32(bf_ih) + f32(bf_hh),
        f32(Wb_ih), f32(Wb_hh), f32(bb_ih) + f32(bb_hh),
        f32(Wd_ih), f32(Wd_hh), f32(bd_ih) + f32(bd_hh),
        f32(w_att), f32(W_out), f32(b_out))
    nc = _build_nc(NOUT=n_output)

    in_maps = []
    for k in range(NCORES):
        in_maps.append({"wpack": wpack,
                        "xs": _prep_xs(x[k * BL:(k + 1) * BL])})
    cores = list(range(NCORES))

    # warm-up: compiles (NEFF is disk-cached across processes) and primes
    # the transfer path; not part of the reported execution time
    res = None
    _tw0 = time.time()
    for attempt in range(3):
        try:
            res = run_bass_kernel_spmd(nc, in_maps, cores)
            break
        except Exception:
            if attempt == 2:
                raise
            time.sleep(2.0)
    warm_ns = int((time.time() - _tw0) * 1e9)

    # timed steady-state execution
    global LAST_EXEC_NS
    try:
        _t0 = time.time()
        res2 = run_bass_kernel_spmd(nc, in_maps, cores)
        LAST_EXEC_NS = int((time.time() - _t0) * 1e9)
        res = res2
    except Exception:
        LAST_EXEC_NS = warm_ns

    ys = np.empty((B, n_output, EMB), np.float32)
    for k in range(NCORES):
        o = np.asarray(res.results[k]["out"], dtype=np.float32)
        ys[k * BL:(k + 1) * BL] = o.reshape(
            EMB, n_output, BL).transpose(2, 1, 0)
    return ys


# revision 5
# speedup vs baseline: 1.9376x; 1.0444x over previous
"""AttentionRNN Trainium2 kernel -- 8-core data-parallel, full on-device model.

Batch (2048) is sharded 8 ways (256 rows/core).  Each core runs the ENTIRE
model on device via one Bass/Tile program:

  embedding lookup   : one-hot trick -- x broadcast (K=1 ones matmul) ->
                       iota compare (DVE) -> table matmul (K=128) against a
                       host-precomputed (emb @ W_ih.T + b) table
  BiLSTM             : 256 fwd + 256 bwd steps packed into shared [*, 512]
                       ops (fwd cols 0:256, bwd 256:512); gate order
                       permuted to i,f,o,g so sigmoid/tanh slices are
                       contiguous; h stored (bf16) 4-steps-per-partition-
                       block: h[s,b,k] at [32*(s%4)+k, (s//4)*256+b]
  attention          : scores via per-partition weight multiply + [128->4]
                       ones matmul; softmax normalization deferred (exp /
                       colsum-Z applied after the context reduction --
                       softmax is shift-invariant wrt the decoder-state
                       term so alpha is decoder-independent and computed
                       once); alpha replicated to the h layout with a fixed
                       [4,128] selector matmul; context = multiply +
                       grouped free reduce + [128->64] combiner matmul
  decoder            : 10 steps; z = Wd_cx@ctx (+bias folded via ones row)
                       + Wd_py@py + Wd_hh@h accumulated in PSUM; output
                       projection W_out/b_out folded the same way; py
                       written straight into the output tile

Two environment workarounds baked in:
  * this walrus build accepts a single sync-wait per instruction, so a BIR
    post-pass splits multi-wait instructions into single-wait NoOps + op
    (installed by monkeypatching bass2jax._decompress_ant_bir);
  * matmul operands/outputs at base partition 32 crash the runtime, so all
    matmuls use base-0 operands (x rows streamed as separate tensors, the
    context partition-combine done as one accumulation group with a
    [128,128] block selector).

kernel() does one untimed warm-up call (compile; NEFF is disk-cached) and
reports LAST_EXEC_NS as the wall time of the subsequent steady-state call.
"""

import numpy as np
from contextlib import ExitStack

EMB = 128
H = 32
VOC = 128
BL = 256
NCORES = 8
B = 2048
S = 256
PERM = np.r_[0:64, 96:128, 64:96]  # gate order i,f,o,g (from i,f,g,o)
LAST_EXEC_NS = 0


def _mk_woff():
    shapes = [("tabf", 128, 128), ("tabb", 128, 128), ("whhf", 32, 128),
              ("whhb", 32, 128), ("wdpy", 128, 128), ("wdcx", 65, 128),
              ("wdhh", 32, 128), ("wout", 33, 128), ("w4", 128, 2),
              ("iotaf", 128, 1), ("cmb2", 128, 128), ("onesg", 128, 4),
              ("sel4b", 4, 128)]
    off, table = 0, {}
    for name, r, c in shapes:
        table[name] = (r, c, off)
        off += c
    return table, off


WOFF, WCOLS = _mk_woff()


def _prep_weights(emb, Wf_ih, Wf_hh, bf, Wb_ih, Wb_hh, bb,
                  Wd_ih, Wd_hh, bd, w_att, W_out, b_out):
    f = lambda a: np.ascontiguousarray(a, dtype=np.float32)
    parts = {}
    parts["tabf"] = f((emb @ Wf_ih.T + bf)[:, PERM])
    parts["tabb"] = f((emb @ Wb_ih.T + bb)[:, PERM])
    parts["whhf"] = f(Wf_hh.T[:, PERM])
    parts["whhb"] = f(Wb_hh.T[:, PERM])
    parts["w4"] = f(np.stack([np.tile(w_att[H:2 * H], 4),
                              np.tile(w_att[2 * H:3 * H], 4)], axis=1))
    parts["wdpy"] = f(Wd_ih[PERM, :EMB].T)
    parts["wdcx"] = f(np.concatenate([Wd_ih[PERM, EMB:].T,
                                      bd[PERM][None, :]], axis=0))
    parts["wdhh"] = f(Wd_hh[PERM].T)
    parts["wout"] = f(np.concatenate([W_out.T, b_out[None, :]], axis=0))
    parts["iotaf"] = np.arange(128, dtype=np.float32)[:, None]
    ones4 = (np.arange(128)[:, None] % 32
             == np.arange(32)[None, :]).astype(np.float32)
    z32 = np.zeros((128, 32), np.float32)
    parts["cmb2"] = np.concatenate([ones4, z32, z32, ones4], axis=1)
    parts["onesg"] = (np.arange(128)[:, None] // 32
                      == np.arange(4)[None, :]).astype(np.float32)
    parts["sel4b"] = (np.arange(4)[:, None]
                      == np.arange(128)[None, :] // 32).astype(np.float32)
    wpack = np.zeros((128, WCOLS), np.float32)
    for name, (rows, cols, off) in WOFF.items():
        wpack[:rows, off:off + cols] = parts[name]
    return wpack


def _prep_xs(x_core):
    import ml_dtypes
    xs2 = np.empty((2, S * BL), np.float32)
    xs2[0] = x_core.T.reshape(-1)
    xs2[1] = x_core[:, ::-1].T.reshape(-1)
    return xs2.astype(ml_dtypes.bfloat16)


def _install_birpatch():
    """Split multi-wait instructions: this walrus accepts one sync-wait per
    instruction, so hoist extras onto single-wait NoOps inserted before it
    on the same engine queue (sequencers execute in order -- equivalent)."""
    import orjson
    from concourse import bass2jax
    if getattr(bass2jax._decompress_ant_bir, "_waitsplit", False):
        return
    orig = bass2jax._decompress_ant_bir
    counter = [0]

    def _split_block(bb):
        out = []
        for ins in bb.get("instructions", []):
            si = ins.get("sync_info") or {}
            waits = si.get("on_wait") or []
            if len(waits) > 1:
                for wx in waits[:-1]:
                    counter[0] += 1
                    out.append({"name": f"I-WSPL{counter[0]}",
                                "opcode": "NoOp",
                                "engine": ins.get("engine"),
                                "ins": [], "outs": [],
                                "debug": ins.get("debug", 0),
                                "sync_info": {"on_wait": [wx],
                                              "on_update": []}})
                si["on_wait"] = [waits[-1]]
            out.append(ins)
        bb["instructions"] = out
        for sub in bb.get("blocks", []) or []:
            _split_block(sub)

    def patched(ant_bir_value):
        raw = orig(ant_bir_value)
        try:
            counter[0] = 0
            bir = orjson.loads(raw)
            for fn in bir.get("functions", []):
                for bb in fn.get("blocks", []) or []:
                    _split_block(bb)
            return orjson.dumps(bir)
        except Exception:
            return raw

    patched._waitsplit = True
    bass2jax._decompress_ant_bir = patched


def _build_nc(NOUT=10):
    import concourse.bass as bass
    import concourse.tile as tile
    from concourse import mybir

    f32 = mybir.dt.float32
    bf16 = mybir.dt.bfloat16
    NB = S // 4
    NTHI = NB
    CHUNK = 8
    AF = mybir.ActivationFunctionType

    nc = bass.Bass()
    d_xs = nc.declare_dram_parameter("xs", [2, S * BL], bf16, isOutput=False)
    d_wp = nc.declare_dram_parameter("wpack", [128, WCOLS], f32,
                                     isOutput=False)
    d_out = nc.declare_dram_parameter("out", [EMB, NOUT * BL], bf16,
                                      isOutput=True)

    with ExitStack() as ctx:
        tc = ctx.enter_context(tile.TileContext(nc))
        const = ctx.enter_context(tc.tile_pool(name="const", bufs=1))
        state = ctx.enter_context(tc.tile_pool(name="state", bufs=1))
        work = ctx.enter_context(tc.tile_pool(name="work", bufs=2))
        xpool = ctx.enter_context(tc.tile_pool(name="xpool", bufs=2))

        wpack = const.tile([128, WCOLS], f32)
        nc.gpsimd.dma_start(wpack[:, :], d_wp[:, :])
        w = {name: wpack[0:r, off:off + c]
             for name, (r, c, off) in WOFF.items()}
        iota_f = w["iotaf"]
        ones_r = const.tile([1, 128], f32)
        nc.vector.memset(ones_r, 1.0)
        ones_b1 = const.tile([1, 128], bf16)
        nc.vector.memset(ones_b1, 1.0)
        ones_cf = const.tile([128, 1], f32)
        nc.vector.memset(ones_cf, 1.0)
        onesg_b = const.tile([128, 4], bf16)
        nc.vector.tensor_copy(onesg_b, w["onesg"])
        sel4b_b = const.tile([4, 128], bf16)
        nc.vector.tensor_copy(sel4b_b, w["sel4b"])
        wdpy_b = const.tile([128, 128], bf16)
        nc.vector.tensor_copy(wdpy_b, w["wdpy"])

        # PE pre-touch of wpack: keeps later matmuls at one wait each
        # (LDWEIGHTS carries a single wait slot).  Pool stays open so the
        # PSUM bank is never reused (reuse would add a bank-WAW wait).
        ps_warm = ctx.enter_context(
            tc.tile_pool(name="ps_warm", bufs=1, space="PSUM"))
        warm = ps_warm.tile([1, 1], f32)
        nc.tensor.matmul(warm, wpack[0:1, 0:1], wpack[0:1, 0:1],
                         start=True, stop=True)

        # ---- scan state ----
        hT2 = state.tile([32, 2 * BL], f32)
        nc.vector.memset(hT2, 0.0)
        c2 = state.tile([32, 2 * BL], f32)
        nc.vector.memset(c2, 0.0)
        hf4 = state.tile([128, NB * BL], bf16)
        hb4 = state.tile([128, NB * BL], bf16)

        AL = mybir.AluOpType

        with tc.tile_pool(name="ps_scan", bufs=2, space="PSUM") as ps_scan, \
             tc.tile_pool(name="ps_scan2", bufs=2, space="PSUM") as ps_scan2:
            CHX = 16
            xchf = xchb = None
            for t in range(S):
                sb_ = S - 1 - t
                if t % CHX == 0:
                    xchf = xpool.tile([1, CHX * BL], bf16, tag="xchf")
                    nc.gpsimd.dma_start(xchf[0:1, :],
                                        d_xs[0:1, t * BL:(t + CHX) * BL])
                    xchb = xpool.tile([1, CHX * BL], bf16, tag="xchb")
                    nc.gpsimd.dma_start(xchb[0:1, :],
                                        d_xs[1:2, t * BL:(t + CHX) * BL])
                lt = t % CHX
                psx = ps_scan.tile([128, 2 * BL], f32, tag="psx")
                nc.tensor.matmul(psx[:, 0:BL], ones_b1,
                                 xchf[0:1, lt * BL:(lt + 1) * BL],
                                 start=True, stop=True)
                nc.tensor.matmul(psx[:, BL:2 * BL], ones_b1,
                                 xchb[0:1, lt * BL:(lt + 1) * BL],
                                 start=True, stop=True)
                oh = work.tile([128, 2 * BL], f32, tag="oh")
                nc.vector.tensor_scalar(oh, psx, iota_f, None,
                                        op0=AL.is_equal)

                psz = ps_scan2.tile([128, 2 * BL], f32, tag="psz")
                nc.tensor.matmul(psz[:, 0:BL], w["tabf"], oh[:, 0:BL],
                                 start=True, stop=False)
                nc.tensor.matmul(psz[:, 0:BL], w["whhf"], hT2[:, 0:BL],
                                 start=False, stop=True)
                nc.tensor.matmul(psz[:, BL:2 * BL], w["tabb"],
                                 oh[:, BL:2 * BL], start=True, stop=False)
                nc.tensor.matmul(psz[:, BL:2 * BL], w["whhb"],
                                 hT2[:, BL:2 * BL], start=False, stop=True)

                sgi = work.tile([32, 2 * BL], f32, tag="sgi")
                nc.scalar.activation(sgi, psz[0:32, :], AF.Sigmoid)
                sgf = work.tile([32, 2 * BL], f32, tag="sgf")
                nc.scalar.activation(sgf, psz[32:64, :], AF.Sigmoid)
                sgo = work.tile([32, 2 * BL], f32, tag="sgo")
                nc.scalar.activation(sgo, psz[64:96, :], AF.Sigmoid)
                tg = work.tile([32, 2 * BL], f32, tag="tg")
                nc.scalar.activation(tg, psz[96:128, :], AF.Tanh)
                t1 = work.tile([32, 2 * BL], f32, tag="t1")
                nc.vector.tensor_mul(t1, sgi, tg)
                nc.vector.tensor_mul(c2, sgf, c2)
                nc.vector.tensor_add(c2, c2, t1)
                tnc = work.tile([32, 2 * BL], f32, tag="tnc")
                nc.scalar.activation(tnc, c2, AF.Tanh)
                nc.vector.tensor_mul(hT2, sgo, tnc)

                nc.gpsimd.tensor_copy(
                    hf4[32 * (t % 4):32 * (t % 4) + 32,
                        (t // 4) * BL:(t // 4) * BL + BL], hT2[:, 0:BL])
                nc.gpsimd.tensor_copy(
                    hb4[32 * (sb_ % 4):32 * (sb_ % 4) + 32,
                        (sb_ // 4) * BL:(sb_ // 4) * BL + BL],
                    hT2[:, BL:2 * BL])

        # ---- attention ----
        exp4 = state.tile([4, NB * BL], bf16)
        ctxT = state.tile([65, BL], f32)
        nc.vector.memset(ctxT[64:65, :], 1.0)

        NCH = (NB * BL) // 512
        with tc.tile_pool(name="ps_att", bufs=2, space="PSUM") as ps_att, \
             tc.tile_pool(name="ps_att1", bufs=1, space="PSUM") as ps_att1, \
             tc.tile_pool(name="ps_att2", bufs=2, space="PSUM") as ps_att2, \
             tc.tile_pool(name="att_sb", bufs=2) as att_sb, \
             tc.tile_pool(name="att_acc", bufs=1) as att_acc:
            for ch in range(NCH):
                cs = ch * 512
                whf = att_sb.tile([128, 512], bf16, tag="whf")
                nc.vector.tensor_scalar(whf, hf4[:, cs:cs + 512],
                                        w["w4"][:, 0:1], None, op0=AL.mult)
                whb = att_sb.tile([128, 512], bf16, tag="whb")
                nc.vector.tensor_scalar(whb, hb4[:, cs:cs + 512],
                                        w["w4"][:, 1:2], None, op0=AL.mult)
                s4p = ps_att2.tile([4, 512], f32, tag="s4p")
                nc.tensor.matmul(s4p, onesg_b, whf, start=True, stop=False)
                nc.tensor.matmul(s4p, onesg_b, whb, start=False, stop=True)
                nc.scalar.activation(exp4[:, cs:cs + 512], s4p, AF.Exp)

            zpart = att_acc.tile([4, BL], f32)
            nc.vector.tensor_reduce(
                zpart, exp4.rearrange("p (l b) -> p b l", l=NB),
                axis=mybir.AxisListType.X, op=AL.add)
            zps = ps_att1.tile([1, BL], f32)
            nc.tensor.matmul(zps, ones_cf[0:4, :], zpart,
                             start=True, stop=True)
            zrec = att_acc.tile([1, BL], f32)
            nc.vector.reciprocal(zrec, zps)

            acc_f = att_acc.tile([128, BL], f32)
            acc_b = att_acc.tile([128, BL], f32)
            for ci in range(NTHI // CHUNK):
                tmpf = att_sb.tile([128, CHUNK * BL], bf16, tag="tmpf")
                tmpb = att_sb.tile([128, CHUNK * BL], bf16, tag="tmpb")
                for li in range(CHUNK):
                    thi = ci * CHUNK + li
                    a4 = ps_att.tile([128, BL], f32, tag="a4")
                    nc.tensor.matmul(a4, sel4b_b,
                                     exp4[:, thi * BL:(thi + 1) * BL],
                                     start=True, stop=True)
                    a4s = att_sb.tile([128, BL], bf16, tag="a4s")
                    nc.scalar.activation(a4s, a4, AF.Copy)
                    nc.vector.tensor_mul(tmpf[:, li * BL:(li + 1) * BL],
                                         hf4[:, thi * BL:(thi + 1) * BL],
                                         a4s)
                    nc.vector.tensor_mul(tmpb[:, li * BL:(li + 1) * BL],
                                         hb4[:, thi * BL:(thi + 1) * BL],
                                         a4s)
                for acc, tmp in ((acc_f, tmpf), (acc_b, tmpb)):
                    red = att_sb.tile([128, BL], f32, tag="red")
                    nc.vector.tensor_reduce(
                        red, tmp.rearrange("p (l b) -> p b l", l=CHUNK),
                        axis=mybir.AxisListType.X, op=AL.add)
                    if ci == 0:
                        nc.vector.tensor_copy(acc, red)
                    else:
                        nc.vector.tensor_add(acc, acc, red)

            ctx_ps = ps_att1.tile([64, BL], f32)
            nc.tensor.matmul(ctx_ps, w["cmb2"][:, 0:64], acc_f,
                             start=True, stop=False)
            nc.tensor.matmul(ctx_ps, w["cmb2"][:, 64:128], acc_b,
                             start=False, stop=True)
            zbc = ps_att1.tile([64, BL], f32)
            nc.tensor.matmul(zbc, ones_r[:, 0:64], zrec,
                             start=True, stop=True)
            zbs = att_acc.tile([64, BL], f32)
            nc.vector.tensor_copy(zbs, zbc)
            nc.vector.tensor_mul(ctxT[0:64, :], zbs, ctx_ps)

        # ---- decoder ----
        out_sb = state.tile([EMB, NOUT * BL], bf16)
        hTd = state.tile([33, BL], f32)
        nc.vector.memset(hTd, 0.0)
        nc.vector.memset(hTd[32:33, :], 1.0)
        cd = state.tile([32, BL], f32)
        nc.vector.memset(cd, 0.0)

        with tc.tile_pool(name="ps_dec", bufs=2, space="PSUM") as ps_dec, \
             tc.tile_pool(name="dec_sb", bufs=2) as dec_sb:
            for t in range(NOUT):
                zd = ps_dec.tile([128, BL], f32, tag="zd")
                nc.tensor.matmul(zd, w["wdcx"], ctxT,
                                 start=True, stop=(t == 0))
                if t > 0:
                    nc.tensor.matmul(zd, wdpy_b,
                                     out_sb[:, (t - 1) * BL:t * BL],
                                     start=False, stop=False)
                    nc.tensor.matmul(zd, w["wdhh"], hTd[0:32, :],
                                     start=False, stop=True)
                sdi = dec_sb.tile([32, BL], f32, tag="sdi")
                nc.scalar.activation(sdi, zd[0:32, :], AF.Sigmoid)
                sdf = dec_sb.tile([32, BL], f32, tag="sdf")
                nc.scalar.activation(sdf, zd[32:64, :], AF.Sigmoid)
                sdo = dec_sb.tile([32, BL], f32, tag="sdo")
                nc.scalar.activation(sdo, zd[64:96, :], AF.Sigmoid)
                tgd = dec_sb.tile([32, BL], f32, tag="tgd")
                nc.scalar.activation(tgd, zd[96:128, :], AF.Tanh)
                t1d = dec_sb.tile([32, BL], f32, tag="t1d")
                nc.vector.tensor_mul(t1d, sdi, tgd)
                if t > 0:
                    nc.vector.tensor_mul(cd, sdf, cd)
                    nc.vector.tensor_add(cd, cd, t1d)
                else:
                    nc.vector.tensor_copy(cd, t1d)
                tncd = dec_sb.tile([32, BL], f32, tag="tncd")
                nc.scalar.activation(tncd, cd, AF.Tanh)
                nc.vector.tensor_mul(hTd[0:32, :], sdo, tncd)
                pyp = ps_dec.tile([128, BL], f32, tag="pyp")
                nc.tensor.matmul(pyp, w["wout"], hTd, start=True, stop=True)
                nc.vector.tensor_copy(out_sb[:, t * BL:(t + 1) * BL], pyp)

        nc.gpsimd.dma_start(d_out[:, :], out_sb[:, :])

    return nc


def kernel(x, n_output, emb, Wf_ih, Wf_hh, bf_ih, bf_hh, Wb_ih, Wb_hh,
           bb_ih, bb_hh, Wd_ih, Wd_hh, bd_ih, bd_hh, w_att, b_att,
           W_out, b_out):
    import os, time
    os.environ["BASS_NEVER_TRACE"] = "1"  # no NTFF hook in this env
    _install_birpatch()
    from concourse.bass_utils import run_bass_kernel_spmd

    x = np.asarray(x)
    n_output = int(n_output)
    f32 = lambda a: np.asarray(a, dtype=np.float32)
    wpack = _prep_weights(
        f32(emb), f32(Wf_ih), f32(Wf_hh), f32(bf_ih) + f32(bf_hh),
        f32(Wb_ih), f32(Wb_hh), f32(bb_ih) + f32(bb_hh),
        f32(Wd_ih), f32(Wd_hh), f32(bd_ih) + f32(bd_hh),
        f32(w_att), f32(W_out), f32(b_out))
    nc = _build_nc(NOUT=n_output)

    in_maps = []
    for k in range(NCORES):
        in_maps.append({"wpack": wpack,
                        "xs": _prep_xs(x[k * BL:(k + 1) * BL])})
    cores = list(range(NCORES))

    # warm-up: compiles (NEFF is disk-cached across processes) and primes
    # the transfer path; not part of the reported execution time
    res = None
    _tw0 = time.time()
    for attempt in range(3):
        try:
            res = run_bass_kernel_spmd(nc, in_maps, cores)
            break
        except Exception:
            if attempt == 2:
                raise
            time.sleep(2.0)
    warm_ns = int((time.time() - _tw0) * 1e9)

    # timed steady-state execution
    global LAST_EXEC_NS
    try:
        _t0 = time.time()
        res2 = run_bass_kernel_spmd(nc, in_maps, cores)
        LAST_EXEC_NS = int((time.time() - _t0) * 1e9)
        res = res2
    except Exception:
        LAST_EXEC_NS = warm_ns

    ys = np.empty((B, n_output, EMB), np.float32)
    for k in range(NCORES):
        o = np.asarray(res.results[k]["out"], dtype=np.float32)
        ys[k * BL:(k + 1) * BL] = o.reshape(
            EMB, n_output, BL).transpose(2, 1, 0)
    return ys
